# revision 8
# baseline (speedup 1.0000x reference)
"""Trainium2 Bass kernel for nn_LocalFeaLowDimDynamicFusion.

Computes, from full (unsharded) inputs:
  embedded_points = W_emb @ sampled_3d_points + b_emb   (B, 256, N)
  local_fea       = W_local @ grid_sample(fea, pts2d) + b_local  (B, 16, N)
with N = N_s * N_anchor * N_y = 32000, B = 4.

Sharding: 8 cores; core i handles batch b = i // 2 and sampling slice
s in [4*(i%2), 4*(i%2)+4) -> 16000 points per core, contiguous in the
output N axis.

Grid sample on device: the feature map is host-repacked into a
"quad-dup" table feaQ[(yp, xp), (e, d, c)] (91*161 positions x 64 f32)
so that one 256B dma_gather element fetches exactly the 4 bilinear
corner vectors of a point. Cell indices and fractional weights are
computed on DVE/ACT; the 16x16 1x1 conv runs on PE after a per-128-point
transpose, with the bias folded in via an appended ones column.
"""

import numpy as np

import concourse.bacc as bacc
import concourse.bass as bass
import concourse.tile as tile
from concourse import mybir
from concourse.bass_utils import run_bass_kernel_spmd
from concourse.library_config import mlp

B, C, H, W = 4, 16, 90, 160
N_S, N_ANCHOR, N_Y = 8, 200, 20
N_FULL = N_S * N_ANCHOR * N_Y  # 32000
N_CORES = 8
NPTS = 16000          # points per core
NJ = NPTS // 128      # 125 j-columns
NPOS = (H + 1) * (W + 1)  # 91 * 161 = 14651 gather positions
ESZ = 64              # f32 per gather element (2x2 corners x 16 ch)
GCH = 1024            # gather indices per dma_gather call (HW ring limit)
JG = 16               # j-columns per pipeline group
ECH = 500             # embed points per matmul chunk

_cache = {}


def _build(inv_scale: float):
    nc = bacc.Bacc("TRN2", target_bir_lowering=False, debug=False,
                   enable_asserts=True, num_devices=N_CORES)
    f32, i32, i16 = mybir.dt.float32, mybir.dt.int32, mybir.dt.int16
    AT = mybir.ActivationFunctionType
    OP = mybir.AluOpType

    feaQ = nc.dram_tensor("feaQ", [NPOS, ESZ], f32, kind="ExternalInput")
    pxw = nc.dram_tensor("pxw", [128, NJ], f32, kind="ExternalInput")
    pyw = nc.dram_tensor("pyw", [128, NJ], f32, kind="ExternalInput")
    pxi = nc.dram_tensor("pxi", [128, NPTS // 16], f32, kind="ExternalInput")
    pyi = nc.dram_tensor("pyi", [128, NPTS // 16], f32, kind="ExternalInput")
    sp5 = nc.dram_tensor("sp5", [5, NPTS], f32, kind="ExternalInput")
    wl17 = nc.dram_tensor("wl17", [17, 16], f32, kind="ExternalInput")
    we5a = nc.dram_tensor("we5a", [5, 128], f32, kind="ExternalInput")
    we5b = nc.dram_tensor("we5b", [5, 128], f32, kind="ExternalInput")
    idm = nc.dram_tensor("idm", [128, 128], f32, kind="ExternalInput")
    emb_out = nc.dram_tensor("emb_out", [256, NPTS], f32, kind="ExternalOutput")
    loc_out = nc.dram_tensor("loc_out", [16, NPTS], f32, kind="ExternalOutput")

    with tile.TileContext(nc) as tc:
        nc.gpsimd.load_library(mlp)
        import contextlib
        with contextlib.ExitStack() as ctx:
            const = ctx.enter_context(tc.tile_pool(name="const", bufs=1))
            idxp = ctx.enter_context(tc.tile_pool(name="idxp", bufs=1))
            wp = ctx.enter_context(tc.tile_pool(name="wp", bufs=1))
            gpool = ctx.enter_context(tc.tile_pool(name="g", bufs=2))
            work = ctx.enter_context(tc.tile_pool(name="work", bufs=2))
            sbt = ctx.enter_context(tc.tile_pool(name="sbt", bufs=3))
            ebuf = ctx.enter_context(tc.tile_pool(name="ebuf", bufs=3))
            pst_p = ctx.enter_context(tc.tile_pool(name="pst", bufs=2, space="PSUM"))
            psc_p = ctx.enter_context(tc.tile_pool(name="psc", bufs=2, space="PSUM"))
            pse_p = ctx.enter_context(tc.tile_pool(name="pse", bufs=4, space="PSUM"))

            # ---- constants / inputs to SBUF ----
            ident = const.tile([128, 128], f32)
            nc.sync.dma_start(out=ident[:], in_=idm.ap())
            wlt = const.tile([17, 16], f32)
            nc.sync.dma_start(out=wlt[:], in_=wl17.ap())
            wea = const.tile([5, 128], f32)
            nc.sync.dma_start(out=wea[:], in_=we5a.ap())
            web = const.tile([5, 128], f32)
            nc.sync.dma_start(out=web[:], in_=we5b.ap())
            pxwt = const.tile([128, NJ], f32)
            nc.sync.dma_start(out=pxwt[:], in_=pxw.ap())
            pywt = const.tile([128, NJ], f32)
            nc.sync.dma_start(out=pywt[:], in_=pyw.ap())
            pxit = const.tile([128, NPTS // 16], f32)
            nc.sync.dma_start(out=pxit[:], in_=pxi.ap())
            pyit = const.tile([128, NPTS // 16], f32)
            nc.sync.dma_start(out=pyit[:], in_=pyi.ap())

            # ---- gather index computation (replicated 16-wrap layout) ----
            nw = NPTS // 16  # 1000
            ax = idxp.tile([128, nw], f32)
            nc.scalar.activation(ax[:], pxit[:], AT.Copy, bias=0.0, scale=inv_scale)
            ay = idxp.tile([128, nw], f32)
            nc.scalar.activation(ay[:], pyit[:], AT.Copy, bias=0.0, scale=inv_scale)
            xi32 = idxp.tile([128, nw], i32)
            nc.vector.tensor_copy(xi32[:], ax[:])   # round-to-nearest = floor cell
            yi32 = idxp.tile([128, nw], i32)
            nc.vector.tensor_copy(yi32[:], ay[:])
            idx32 = idxp.tile([128, nw], i32)
            nc.vector.scalar_tensor_tensor(idx32[:], yi32[:], W + 1, xi32[:],
                                           OP.mult, OP.add)
            idx16 = idxp.tile([128, nw], i16)
            nc.vector.tensor_copy(idx16[:], idx32[:])

            # ---- bilinear weights (block layout [p, j] = point j*128+p) ----
            axw = wp.tile([128, NJ], f32)
            nc.scalar.activation(axw[:], pxwt[:], AT.Copy, bias=0.0, scale=inv_scale)
            ayw = wp.tile([128, NJ], f32)
            nc.scalar.activation(ayw[:], pywt[:], AT.Copy, bias=0.0, scale=inv_scale)
            xiw = wp.tile([128, NJ], i32)
            nc.vector.tensor_copy(xiw[:], axw[:])
            yiw = wp.tile([128, NJ], i32)
            nc.vector.tensor_copy(yiw[:], ayw[:])
            xif = wp.tile([128, NJ], f32)
            nc.vector.tensor_copy(xif[:], xiw[:])
            yif = wp.tile([128, NJ], f32)
            nc.vector.tensor_copy(yif[:], yiw[:])
            # fxp = (x - xi) - 0.5 in [-1, 1]; weights wx1 = fxp + 0.5, wx0 = 0.5 - fxp
            fxp = wp.tile([128, NJ], f32)
            nc.vector.scalar_tensor_tensor(fxp[:], xif[:], -1.0, axw[:],
                                           OP.mult, OP.add)
            fyp = wp.tile([128, NJ], f32)
            nc.vector.scalar_tensor_tensor(fyp[:], yif[:], -1.0, ayw[:],
                                           OP.mult, OP.add)
            wx = wp.tile([128, NJ, 2], f32)
            nc.vector.tensor_scalar(wx[:, :, 0], fxp[:], -1.0, 0.5, OP.mult, OP.add)
            nc.vector.tensor_scalar(wx[:, :, 1], fxp[:], 0.5, None, OP.add)
            wy = wp.tile([128, NJ, 2], f32)
            nc.vector.tensor_scalar(wy[:, :, 0], fyp[:], -1.0, 0.5, OP.mult, OP.add)
            nc.vector.tensor_scalar(wy[:, :, 1], fyp[:], 0.5, None, OP.add)
            # s4w[p, j, k] with k = e*2 + d (x-corner major, y fast)
            s4w = wp.tile([128, NJ, 4], f32)
            for e in range(2):
                for d in range(2):
                    nc.vector.tensor_tensor(s4w[:, :, e * 2 + d], wx[:, :, e],
                                            wy[:, :, d], OP.mult)

            # smp: bilinear result + ones column for conv bias folding
            smp = wp.tile([128, NJ, 17], f32)
            nc.vector.memset(smp[:, :, 16], 1.0)

            # ---- main pipeline over groups of JG j-columns ----
            n_groups = (NJ + JG - 1) // JG
            for g in range(n_groups):
                j0 = g * JG
                jn = min(JG, NJ - j0)
                gt = gpool.tile([128, JG, ESZ], f32, tag="G")
                # gather this group's points (j*128+p for j in [j0, j0+jn))
                i0 = j0 * 128
                ni = jn * 128
                for c0 in range(0, ni, GCH):
                    cn = min(GCH, ni - c0)
                    nc.gpsimd.dma_gather(
                        gt[:, c0 // 128 : (c0 + cn) // 128, :],
                        feaQ.ap(),
                        idx16[:, (i0 + c0) // 16 : (i0 + c0 + cn) // 16],
                        cn, cn, ESZ)
                # weight and reduce corners
                sexp = work.tile([128, JG, 4, 16], f32, tag="sexp")
                src_b = bass.AP(
                    tensor=s4w.tensor, offset=s4w[:, j0, 0].offset,
                    ap=[s4w.ap[0], [4, jn], [1, 4], [0, 16]])
                nc.vector.tensor_copy(sexp[:, :jn], src_b)
                gw = work.tile([128, JG, 4, 16], f32, tag="gw")
                nc.vector.tensor_tensor(
                    gw[:, :jn], gt[:, :jn].rearrange("p j (k c) -> p j k c", k=4),
                    sexp[:, :jn], OP.mult)
                u2 = work.tile([128, JG, 2, 16], f32, tag="u2")
                nc.vector.tensor_tensor(u2[:, :jn], gw[:, :jn, 0:2, :],
                                        gw[:, :jn, 2:4, :], OP.add)
                nc.vector.tensor_tensor(smp[:, j0:j0 + jn, 0:16], u2[:, :jn, 0, :],
                                        u2[:, :jn, 1, :], OP.add)
                # conv: per-j transpose + matmul, 4 j share one psum bank
                for jj in range(0, jn, 4):
                    pc = psc_p.tile([16, 512], f32, tag="pc")
                    m = min(4, jn - jj)
                    for q in range(m):
                        j = j0 + jj + q
                        pt = pst_p.tile([17, 128], f32, tag="pt")
                        nc.tensor.transpose(pt[:], smp[:, j, :], ident[:])
                        st = sbt.tile([17, 128], f32, tag="st")
                        nc.vector.tensor_copy(st[:], pt[:])
                        nc.tensor.matmul(pc[:, q * 128:(q + 1) * 128], wlt[:], st[:])
                    lc = sbt.tile([16, 512], f32, tag="lc")
                    nc.scalar.activation(lc[:, : m * 128], pc[:, : m * 128],
                                         AT.Copy, bias=0.0, scale=1.0)
                    nc.sync.dma_start(
                        out=loc_out.ap()[:, (j0 + jj) * 128 : (j0 + jj + m) * 128],
                        in_=lc[:, : m * 128])

            # ---- embed branch ----
            for c in range(0, NPTS, ECH):
                cn = min(ECH, NPTS - c)
                spc = ebuf.tile([5, ECH], f32, tag="spc")
                nc.sync.dma_start(out=spc[:, :cn], in_=sp5.ap()[:, c:c + cn])
                pa = pse_p.tile([128, ECH], f32, tag="pe")
                nc.tensor.matmul(pa[:, :cn], wea[:], spc[:, :cn])
                ea = ebuf.tile([128, ECH], f32, tag="ea")
                nc.vector.tensor_copy(ea[:, :cn], pa[:, :cn])
                nc.sync.dma_start(out=emb_out.ap()[0:128, c:c + cn], in_=ea[:, :cn])
                pb = pse_p.tile([128, ECH], f32, tag="pe")
                nc.tensor.matmul(pb[:, :cn], web[:], spc[:, :cn])
                eb = ebuf.tile([128, ECH], f32, tag="eb")
                nc.scalar.activation(eb[:, :cn], pb[:, :cn], AT.Copy,
                                     bias=0.0, scale=1.0)
                nc.sync.dma_start(out=emb_out.ap()[128:256, c:c + cn], in_=eb[:, :cn])

    nc.compile()
    return nc


def _prep_inputs(fea, sampled_3d_points, points_2d_coord, scale):
    """Host-side sharding / layout prep. Returns per-core input dicts."""
    from numpy.lib.stride_tricks import sliding_window_view

    fea = np.asarray(fea, dtype=np.float32)
    sp = np.asarray(sampled_3d_points, dtype=np.float32)
    p2 = np.asarray(points_2d_coord, dtype=np.float32)

    # quad-dup gather tables, one per batch
    feaQs = []
    for b in range(B):
        fp = np.zeros((H + 2, W + 2, C), np.float32)
        fp[1:H + 1, 1:W + 1, :] = fea[b].transpose(1, 2, 0)
        sw = sliding_window_view(fp, (2, 2), axis=(0, 1))  # [91+1.., 161.., c, d, e]
        q = sw[:H + 1, :W + 1].transpose(0, 1, 4, 3, 2)    # [yp, xp, e, d, c]
        feaQs.append(np.ascontiguousarray(q).reshape(NPOS, ESZ))

    in_maps = []
    for core in range(N_CORES):
        b = core // 2
        s0 = 4 * (core % 2)
        pts = p2[s0:s0 + 4, b].reshape(NPTS, 2)
        px = np.ascontiguousarray(pts[:, 0])
        py = np.ascontiguousarray(pts[:, 1])
        spc = sp[s0:s0 + 4, b].reshape(NPTS, 4)
        sp5 = np.empty((5, NPTS), np.float32)
        sp5[0:4] = spc.T
        sp5[4] = 1.0
        in_maps.append({
            "feaQ": feaQs[b],
            "pxw": np.ascontiguousarray(px.reshape(NJ, 128).T),
            "pyw": np.ascontiguousarray(py.reshape(NJ, 128).T),
            "pxi": np.ascontiguousarray(np.tile(px.reshape(-1, 16).T, (8, 1))),
            "pyi": np.ascontiguousarray(np.tile(py.reshape(-1, 16).T, (8, 1))),
            "sp5": sp5,
        })
    return in_maps


def kernel(fea, sampled_3d_points, points_2d_coord, W_local, b_local,
           W_emb, b_emb, scale):
    scale = float(np.asarray(scale))
    key = scale
    if key not in _cache:
        _cache[key] = _build(1.0 / scale)
    nc = _cache[key]

    W_local = np.asarray(W_local, dtype=np.float32)
    b_local = np.asarray(b_local, dtype=np.float32)
    W_emb = np.asarray(W_emb, dtype=np.float32)
    b_emb = np.asarray(b_emb, dtype=np.float32)

    wl17 = np.vstack([W_local.T, b_local[None, :]]).astype(np.float32)  # [17, 16]
    we5 = np.vstack([W_emb.T, b_emb[None, :]]).astype(np.float32)       # [5, 256]
    ident = np.eye(128, dtype=np.float32)

    in_maps = _prep_inputs(fea, sampled_3d_points, points_2d_coord, scale)
    for m in in_maps:
        m["wl17"] = wl17
        m["we5a"] = np.ascontiguousarray(we5[:, :128])
        m["we5b"] = np.ascontiguousarray(we5[:, 128:])
        m["idm"] = ident

    res = run_bass_kernel_spmd(nc, in_maps, core_ids=list(range(N_CORES)))

    embedded = np.empty((B, 256, N_FULL), np.float32)
    local = np.empty((B, 16, N_FULL), np.float32)
    for core in range(N_CORES):
        b = core // 2
        s0 = 4 * (core % 2)
        sl = slice(s0 * (N_ANCHOR * N_Y), (s0 + 4) * (N_ANCHOR * N_Y))
        embedded[b, :, sl] = res.results[core]["emb_out"]
        local[b, :, sl] = res.results[core]["loc_out"]
    return embedded, local


# revision 10
# speedup vs baseline: 1.3609x; 1.3609x over previous
"""Trainium2 Bass kernel for nn_LocalFeaLowDimDynamicFusion.

Computes, from full (unsharded) inputs:
  embedded_points = W_emb @ sampled_3d_points + b_emb   (B, 256, N)
  local_fea       = W_local @ grid_sample(fea, pts2d) + b_local  (B, 16, N)
with N = N_s * N_anchor * N_y = 32000, B = 4.

Sharding: 8 cores; core i handles batch b = i // 2 and sampling slice
s in [4*(i%2), 4*(i%2)+4) -> 16000 points per core, contiguous in the
output N axis.

Grid sample on device: the feature map is host-repacked into a
"quad-dup" table feaQ[(yp, xp), (e, d, c)] (91*161 positions x 64 f32)
so that one 256B dma_gather element fetches exactly the 4 bilinear
corner vectors of a point. Cell indices and fractional weights are
computed on DVE/ACT; the 16x16 1x1 conv runs on PE after a per-128-point
transpose, with the bias folded in via an appended ones column.
"""

import numpy as np

import concourse.bacc as bacc
import concourse.bass as bass
import concourse.tile as tile
from concourse import mybir
from concourse.bass_utils import run_bass_kernel_spmd
from concourse.library_config import mlp

B, C, H, W = 4, 16, 90, 160
N_S, N_ANCHOR, N_Y = 8, 200, 20
N_FULL = N_S * N_ANCHOR * N_Y  # 32000
N_CORES = 8
NPTS = 16000          # points per core
NJ = NPTS // 128      # 125 j-columns
NPOS = (H + 1) * (W + 1)  # 91 * 161 = 14651 gather positions
ESZ = 64              # f32 per gather element (2x2 corners x 16 ch)
GCH = 1024            # gather indices per dma_gather call (HW ring limit)
JG = 16               # j-columns per pipeline group
ECH = 500             # embed points per matmul chunk

_cache = {}


def _build(inv_scale: float):
    nc = bacc.Bacc("TRN2", target_bir_lowering=False, debug=False,
                   enable_asserts=True, num_devices=N_CORES)
    f32, i32, i16 = mybir.dt.float32, mybir.dt.int32, mybir.dt.int16
    AT = mybir.ActivationFunctionType
    OP = mybir.AluOpType

    feaQ = nc.dram_tensor("feaQ", [NPOS, ESZ], f32, kind="ExternalInput")
    pxw = nc.dram_tensor("pxw", [128, NJ], f32, kind="ExternalInput")
    pyw = nc.dram_tensor("pyw", [128, NJ], f32, kind="ExternalInput")
    pxi = nc.dram_tensor("pxi", [128, NPTS // 16], f32, kind="ExternalInput")
    pyi = nc.dram_tensor("pyi", [128, NPTS // 16], f32, kind="ExternalInput")
    sp5 = nc.dram_tensor("sp5", [5, NPTS], f32, kind="ExternalInput")
    wl17 = nc.dram_tensor("wl17", [17, 16], f32, kind="ExternalInput")
    we5a = nc.dram_tensor("we5a", [5, 128], f32, kind="ExternalInput")
    we5b = nc.dram_tensor("we5b", [5, 128], f32, kind="ExternalInput")
    idm = nc.dram_tensor("idm", [128, 128], f32, kind="ExternalInput")
    emb_out = nc.dram_tensor("emb_out", [256, NPTS], f32, kind="ExternalOutput")
    loc_out = nc.dram_tensor("loc_out", [16, NPTS], f32, kind="ExternalOutput")

    with tile.TileContext(nc) as tc:
        nc.gpsimd.load_library(mlp)
        import contextlib
        with contextlib.ExitStack() as ctx:
            const = ctx.enter_context(tc.tile_pool(name="const", bufs=1))
            idxp = ctx.enter_context(tc.tile_pool(name="idxp", bufs=1))
            wp = ctx.enter_context(tc.tile_pool(name="wp", bufs=1))
            gpool = ctx.enter_context(tc.tile_pool(name="g", bufs=2))
            work = ctx.enter_context(tc.tile_pool(name="work", bufs=2))
            sbt = ctx.enter_context(tc.tile_pool(name="sbt", bufs=3))
            ebuf = ctx.enter_context(tc.tile_pool(name="ebuf", bufs=3))
            pst_p = ctx.enter_context(tc.tile_pool(name="pst", bufs=2, space="PSUM"))
            psc_p = ctx.enter_context(tc.tile_pool(name="psc", bufs=2, space="PSUM"))
            pse_p = ctx.enter_context(tc.tile_pool(name="pse", bufs=4, space="PSUM"))

            # ---- constants / inputs to SBUF ----
            f32r = mybir.dt.float32r
            ident = const.tile([128, 128], f32)
            nc.sync.dma_start(out=ident[:], in_=idm.ap())
            identr = const.tile([128, 128], f32r)
            nc.vector.tensor_copy(identr[:], ident[:])
            wlt = const.tile([17, 16], f32)
            nc.sync.dma_start(out=wlt[:], in_=wl17.ap())
            wltr = const.tile([17, 16], f32r)
            nc.vector.tensor_copy(wltr[:], wlt[:])
            wea = const.tile([5, 128], f32)
            nc.sync.dma_start(out=wea[:], in_=we5a.ap())
            wear = const.tile([5, 128], f32r)
            nc.vector.tensor_copy(wear[:], wea[:])
            web = const.tile([5, 128], f32)
            nc.sync.dma_start(out=web[:], in_=we5b.ap())
            webr = const.tile([5, 128], f32r)
            nc.vector.tensor_copy(webr[:], web[:])
            pxwt = const.tile([128, NJ], f32)
            nc.sync.dma_start(out=pxwt[:], in_=pxw.ap())
            pywt = const.tile([128, NJ], f32)
            nc.sync.dma_start(out=pywt[:], in_=pyw.ap())
            pxit = const.tile([128, NPTS // 16], f32)
            nc.sync.dma_start(out=pxit[:], in_=pxi.ap())
            pyit = const.tile([128, NPTS // 16], f32)
            nc.sync.dma_start(out=pyit[:], in_=pyi.ap())

            # ---- gather index computation (replicated 16-wrap layout) ----
            nw = NPTS // 16  # 1000
            ax = idxp.tile([128, nw], f32)
            nc.scalar.activation(ax[:], pxit[:], AT.Copy, bias=0.0, scale=inv_scale)
            ay = idxp.tile([128, nw], f32)
            nc.scalar.activation(ay[:], pyit[:], AT.Copy, bias=0.0, scale=inv_scale)
            xi32 = idxp.tile([128, nw], i32)
            nc.vector.tensor_copy(xi32[:], ax[:])   # round-to-nearest = floor cell
            yi32 = idxp.tile([128, nw], i32)
            nc.vector.tensor_copy(yi32[:], ay[:])
            idx32 = idxp.tile([128, nw], i32)
            nc.vector.scalar_tensor_tensor(idx32[:], yi32[:], W + 1, xi32[:],
                                           OP.mult, OP.add)
            idx16 = idxp.tile([128, nw], i16)
            nc.vector.tensor_copy(idx16[:], idx32[:])

            # ---- bilinear weights (block layout [p, j] = point j*128+p) ----
            axw = wp.tile([128, NJ], f32)
            nc.scalar.activation(axw[:], pxwt[:], AT.Copy, bias=0.0, scale=inv_scale)
            ayw = wp.tile([128, NJ], f32)
            nc.scalar.activation(ayw[:], pywt[:], AT.Copy, bias=0.0, scale=inv_scale)
            xiw = wp.tile([128, NJ], i32)
            nc.vector.tensor_copy(xiw[:], axw[:])
            yiw = wp.tile([128, NJ], i32)
            nc.vector.tensor_copy(yiw[:], ayw[:])
            xif = wp.tile([128, NJ], f32)
            nc.vector.tensor_copy(xif[:], xiw[:])
            yif = wp.tile([128, NJ], f32)
            nc.vector.tensor_copy(yif[:], yiw[:])
            # fxp = (x - xi) - 0.5 in [-1, 1]; weights wx1 = fxp + 0.5, wx0 = 0.5 - fxp
            fxp = wp.tile([128, NJ], f32)
            nc.vector.scalar_tensor_tensor(fxp[:], xif[:], -1.0, axw[:],
                                           OP.mult, OP.add)
            fyp = wp.tile([128, NJ], f32)
            nc.vector.scalar_tensor_tensor(fyp[:], yif[:], -1.0, ayw[:],
                                           OP.mult, OP.add)
            wx = wp.tile([128, NJ, 2], f32)
            nc.vector.tensor_scalar(wx[:, :, 0], fxp[:], -1.0, 0.5, OP.mult, OP.add)
            nc.vector.tensor_scalar(wx[:, :, 1], fxp[:], 0.5, None, OP.add)
            wy = wp.tile([128, NJ, 2], f32)
            nc.vector.tensor_scalar(wy[:, :, 0], fyp[:], -1.0, 0.5, OP.mult, OP.add)
            nc.vector.tensor_scalar(wy[:, :, 1], fyp[:], 0.5, None, OP.add)
            # s4w[p, j, k] with k = e*2 + d (x-corner major, y fast)
            s4w = wp.tile([128, NJ, 4], f32)
            for e in range(2):
                for d in range(2):
                    nc.vector.tensor_tensor(s4w[:, :, e * 2 + d], wx[:, :, e],
                                            wy[:, :, d], OP.mult)

            # smp: bilinear result + ones column for conv bias folding.
            # float32r + padded to 32 so 4 j-columns transpose as one
            # aligned [128, 128] PE pass.
            smp = wp.tile([128, NJ, 32], f32r)
            ones_f = wp.tile([128, NJ], f32)
            nc.vector.memset(ones_f[:], 1.0)
            nc.vector.tensor_copy(smp[:, :, 16], ones_f[:])

            # ---- main pipeline over groups of JG j-columns ----
            n_groups = (NJ + JG - 1) // JG
            for g in range(n_groups):
                j0 = g * JG
                jn = min(JG, NJ - j0)
                gt = gpool.tile([128, JG, ESZ], f32, tag="G")
                # gather this group's points (j*128+p for j in [j0, j0+jn))
                i0 = j0 * 128
                ni = jn * 128
                for c0 in range(0, ni, GCH):
                    cn = min(GCH, ni - c0)
                    nc.gpsimd.dma_gather(
                        gt[:, c0 // 128 : (c0 + cn) // 128, :],
                        feaQ.ap(),
                        idx16[:, (i0 + c0) // 16 : (i0 + c0 + cn) // 16],
                        cn, cn, ESZ)
                # weight and reduce corners
                sexp = work.tile([128, JG, 4, 16], f32, tag="sexp")
                src_b = bass.AP(
                    tensor=s4w.tensor, offset=s4w[:, j0, 0].offset,
                    ap=[s4w.ap[0], [4, jn], [1, 4], [0, 16]])
                nc.vector.tensor_copy(sexp[:, :jn], src_b)
                gw = work.tile([128, JG, 4, 16], f32, tag="gw")
                nc.vector.tensor_tensor(
                    gw[:, :jn], gt[:, :jn].rearrange("p j (k c) -> p j k c", k=4),
                    sexp[:, :jn], OP.mult)
                u2 = work.tile([128, JG, 2, 16], f32, tag="u2")
                nc.vector.tensor_tensor(u2[:, :jn], gw[:, :jn, 0:2, :],
                                        gw[:, :jn, 2:4, :], OP.add)
                nc.vector.tensor_tensor(smp[:, j0:j0 + jn, 0:16], u2[:, :jn, 0, :],
                                        u2[:, :jn, 1, :], OP.add)
                # conv: 4 j-columns per PE transpose, one f32r matmul at N=512
                for jj in range(0, jn, 4):
                    m = min(4, jn - jj)
                    pt = pst_p.tile([128, 128], f32r, tag="pt")
                    nc.tensor.transpose(
                        pt[: m * 32, :],
                        smp[:, j0 + jj : j0 + jj + m, :].rearrange(
                            "p j c -> p (j c)"),
                        identr[:])
                    stw = sbt.tile([17, 512], f32r, tag="st")
                    for q in range(m):
                        nc.vector.tensor_copy(stw[:, q * 128:(q + 1) * 128],
                                              pt[32 * q : 32 * q + 17, :])
                    pc = psc_p.tile([16, 512], f32, tag="pc")
                    nc.tensor.matmul(pc[:, : m * 128], wltr[:], stw[:, : m * 128])
                    lc = sbt.tile([16, 512], f32, tag="lc")
                    nc.scalar.activation(lc[:, : m * 128], pc[:, : m * 128],
                                         AT.Copy, bias=0.0, scale=1.0)
                    nc.sync.dma_start(
                        out=loc_out.ap()[:, (j0 + jj) * 128 : (j0 + jj + m) * 128],
                        in_=lc[:, : m * 128])

            # ---- embed branch ----
            for c in range(0, NPTS, ECH):
                cn = min(ECH, NPTS - c)
                spc = ebuf.tile([5, ECH], f32, tag="spc")
                nc.sync.dma_start(out=spc[:, :cn], in_=sp5.ap()[:, c:c + cn])
                spcr = ebuf.tile([5, ECH], f32r, tag="spcr")
                nc.vector.tensor_copy(spcr[:, :cn], spc[:, :cn])
                pa = pse_p.tile([128, ECH], f32, tag="pe")
                nc.tensor.matmul(pa[:, :cn], wear[:], spcr[:, :cn])
                ea = ebuf.tile([128, ECH], f32, tag="ea")
                nc.vector.tensor_copy(ea[:, :cn], pa[:, :cn])
                nc.sync.dma_start(out=emb_out.ap()[0:128, c:c + cn], in_=ea[:, :cn])
                pb = pse_p.tile([128, ECH], f32, tag="pe")
                nc.tensor.matmul(pb[:, :cn], webr[:], spcr[:, :cn])
                eb = ebuf.tile([128, ECH], f32, tag="eb")
                nc.scalar.activation(eb[:, :cn], pb[:, :cn], AT.Copy,
                                     bias=0.0, scale=1.0)
                nc.sync.dma_start(out=emb_out.ap()[128:256, c:c + cn], in_=eb[:, :cn])

    nc.compile()
    return nc


def _prep_inputs(fea, sampled_3d_points, points_2d_coord, scale):
    """Host-side sharding / layout prep. Returns per-core input dicts."""
    from numpy.lib.stride_tricks import sliding_window_view

    fea = np.asarray(fea, dtype=np.float32)
    sp = np.asarray(sampled_3d_points, dtype=np.float32)
    p2 = np.asarray(points_2d_coord, dtype=np.float32)

    # quad-dup gather tables, one per batch
    feaQs = []
    for b in range(B):
        fp = np.zeros((H + 2, W + 2, C), np.float32)
        fp[1:H + 1, 1:W + 1, :] = fea[b].transpose(1, 2, 0)
        sw = sliding_window_view(fp, (2, 2), axis=(0, 1))  # [91+1.., 161.., c, d, e]
        q = sw[:H + 1, :W + 1].transpose(0, 1, 4, 3, 2)    # [yp, xp, e, d, c]
        feaQs.append(np.ascontiguousarray(q).reshape(NPOS, ESZ))

    in_maps = []
    for core in range(N_CORES):
        b = core // 2
        s0 = 4 * (core % 2)
        pts = p2[s0:s0 + 4, b].reshape(NPTS, 2)
        px = np.ascontiguousarray(pts[:, 0])
        py = np.ascontiguousarray(pts[:, 1])
        spc = sp[s0:s0 + 4, b].reshape(NPTS, 4)
        sp5 = np.empty((5, NPTS), np.float32)
        sp5[0:4] = spc.T
        sp5[4] = 1.0
        in_maps.append({
            "feaQ": feaQs[b],
            "pxw": np.ascontiguousarray(px.reshape(NJ, 128).T),
            "pyw": np.ascontiguousarray(py.reshape(NJ, 128).T),
            "pxi": np.ascontiguousarray(np.tile(px.reshape(-1, 16).T, (8, 1))),
            "pyi": np.ascontiguousarray(np.tile(py.reshape(-1, 16).T, (8, 1))),
            "sp5": sp5,
        })
    return in_maps


def kernel(fea, sampled_3d_points, points_2d_coord, W_local, b_local,
           W_emb, b_emb, scale):
    scale = float(np.asarray(scale))
    key = scale
    if key not in _cache:
        _cache[key] = _build(1.0 / scale)
    nc = _cache[key]

    W_local = np.asarray(W_local, dtype=np.float32)
    b_local = np.asarray(b_local, dtype=np.float32)
    W_emb = np.asarray(W_emb, dtype=np.float32)
    b_emb = np.asarray(b_emb, dtype=np.float32)

    wl17 = np.vstack([W_local.T, b_local[None, :]]).astype(np.float32)  # [17, 16]
    we5 = np.vstack([W_emb.T, b_emb[None, :]]).astype(np.float32)       # [5, 256]
    ident = np.eye(128, dtype=np.float32)

    in_maps = _prep_inputs(fea, sampled_3d_points, points_2d_coord, scale)
    for m in in_maps:
        m["wl17"] = wl17
        m["we5a"] = np.ascontiguousarray(we5[:, :128])
        m["we5b"] = np.ascontiguousarray(we5[:, 128:])
        m["idm"] = ident

    res = run_bass_kernel_spmd(nc, in_maps, core_ids=list(range(N_CORES)))

    embedded = np.empty((B, 256, N_FULL), np.float32)
    local = np.empty((B, 16, N_FULL), np.float32)
    for core in range(N_CORES):
        b = core // 2
        s0 = 4 * (core % 2)
        sl = slice(s0 * (N_ANCHOR * N_Y), (s0 + 4) * (N_ANCHOR * N_Y))
        embedded[b, :, sl] = res.results[core]["emb_out"]
        local[b, :, sl] = res.results[core]["loc_out"]
    return embedded, local


# revision 12
# speedup vs baseline: 1.5674x; 1.1517x over previous
"""Trainium2 Bass kernel for nn_LocalFeaLowDimDynamicFusion.

Computes, from full (unsharded) inputs:
  embedded_points = W_emb @ sampled_3d_points + b_emb   (B, 256, N)
  local_fea       = W_local @ grid_sample(fea, pts2d) + b_local  (B, 16, N)
with N = N_s * N_anchor * N_y = 32000, B = 4.

Sharding: 8 cores; core i handles batch b = i // 2 and sampling slice
s in [4*(i%2), 4*(i%2)+4) -> 16000 points per core, contiguous in the
output N axis.

Grid sample on device: the feature map is host-repacked into a
"quad-dup" table feaQ[(yp, xp), (e, d, c)] (91*161 positions x 64 f32)
so that one 256B dma_gather element fetches exactly the 4 bilinear
corner vectors of a point. Cell indices and fractional weights are
computed on DVE/ACT; the 16x16 1x1 conv runs on PE after a per-128-point
transpose, with the bias folded in via an appended ones column.
"""

import numpy as np

import concourse.bacc as bacc
import concourse.bass as bass
import concourse.tile as tile
from concourse import mybir
from concourse.bass_utils import run_bass_kernel_spmd
from concourse.library_config import mlp

B, C, H, W = 4, 16, 90, 160
N_S, N_ANCHOR, N_Y = 8, 200, 20
N_FULL = N_S * N_ANCHOR * N_Y  # 32000
N_CORES = 8
NPTS = 16000          # points per core
NJ = NPTS // 128      # 125 j-columns
NPOS = (H + 1) * (W + 1)  # 91 * 161 = 14651 gather positions
ESZ = 64              # f32 per gather element (2x2 corners x 16 ch)
GCH = 512             # gather indices per dma_gather call (ring-limit safe)
JG = 16               # j-columns per pipeline group
ECH = 500             # embed points per matmul chunk

_cache = {}


def _build(inv_scale: float):
    nc = bacc.Bacc("TRN2", target_bir_lowering=False, debug=False,
                   enable_asserts=True, num_devices=N_CORES)
    f32, i32, i16 = mybir.dt.float32, mybir.dt.int32, mybir.dt.int16
    AT = mybir.ActivationFunctionType
    OP = mybir.AluOpType

    feaQ = nc.dram_tensor("feaQ", [NPOS, ESZ], f32, kind="ExternalInput")
    pxw = nc.dram_tensor("pxw", [128, NJ], f32, kind="ExternalInput")
    pyw = nc.dram_tensor("pyw", [128, NJ], f32, kind="ExternalInput")
    pxi = nc.dram_tensor("pxi", [128, NPTS // 16], f32, kind="ExternalInput")
    pyi = nc.dram_tensor("pyi", [128, NPTS // 16], f32, kind="ExternalInput")
    sp5 = nc.dram_tensor("sp5", [5, NPTS], f32, kind="ExternalInput")
    wl17 = nc.dram_tensor("wl17", [17, 16], f32, kind="ExternalInput")
    we5a = nc.dram_tensor("we5a", [5, 128], f32, kind="ExternalInput")
    we5b = nc.dram_tensor("we5b", [5, 128], f32, kind="ExternalInput")
    idm = nc.dram_tensor("idm", [128, 128], f32, kind="ExternalInput")
    emb_out = nc.dram_tensor("emb_out", [256, NPTS], f32, kind="ExternalOutput")
    loc_out = nc.dram_tensor("loc_out", [16, NPTS], f32, kind="ExternalOutput")

    with tile.TileContext(nc) as tc:
        nc.gpsimd.load_library(mlp)
        import contextlib
        with contextlib.ExitStack() as ctx:
            const = ctx.enter_context(tc.tile_pool(name="const", bufs=1))
            idxp = ctx.enter_context(tc.tile_pool(name="idxp", bufs=1))
            wp = ctx.enter_context(tc.tile_pool(name="wp", bufs=1))
            gpool = ctx.enter_context(tc.tile_pool(name="g", bufs=2))
            work = ctx.enter_context(tc.tile_pool(name="work", bufs=2))
            sbt = ctx.enter_context(tc.tile_pool(name="sbt", bufs=3))
            ebuf = ctx.enter_context(tc.tile_pool(name="ebuf", bufs=3))
            pst_p = ctx.enter_context(tc.tile_pool(name="pst", bufs=2, space="PSUM"))
            psc_p = ctx.enter_context(tc.tile_pool(name="psc", bufs=2, space="PSUM"))
            pse_p = ctx.enter_context(tc.tile_pool(name="pse", bufs=4, space="PSUM"))

            # ---- constants / inputs to SBUF ----
            f32r = mybir.dt.float32r
            ident = const.tile([128, 128], f32)
            nc.sync.dma_start(out=ident[:], in_=idm.ap())
            identr = const.tile([128, 128], f32r)
            nc.vector.tensor_copy(identr[:], ident[:])
            wlt = const.tile([17, 16], f32)
            nc.sync.dma_start(out=wlt[:], in_=wl17.ap())
            wltr = const.tile([17, 16], f32r)
            nc.vector.tensor_copy(wltr[:], wlt[:])
            wea = const.tile([5, 128], f32)
            nc.sync.dma_start(out=wea[:], in_=we5a.ap())
            wear = const.tile([5, 128], f32r)
            nc.vector.tensor_copy(wear[:], wea[:])
            web = const.tile([5, 128], f32)
            nc.sync.dma_start(out=web[:], in_=we5b.ap())
            webr = const.tile([5, 128], f32r)
            nc.vector.tensor_copy(webr[:], web[:])
            pxwt = const.tile([128, NJ], f32)
            nc.sync.dma_start(out=pxwt[:], in_=pxw.ap())
            pywt = const.tile([128, NJ], f32)
            nc.sync.dma_start(out=pywt[:], in_=pyw.ap())
            pxit = const.tile([128, NPTS // 16], f32)
            nc.sync.dma_start(out=pxit[:], in_=pxi.ap())
            pyit = const.tile([128, NPTS // 16], f32)
            nc.sync.dma_start(out=pyit[:], in_=pyi.ap())

            # ---- gather index computation (replicated 16-wrap layout) ----
            nw = NPTS // 16  # 1000
            ax = idxp.tile([128, nw], f32)
            nc.scalar.activation(ax[:], pxit[:], AT.Copy, bias=0.0, scale=inv_scale)
            ay = idxp.tile([128, nw], f32)
            nc.scalar.activation(ay[:], pyit[:], AT.Copy, bias=0.0, scale=inv_scale)
            xi32 = idxp.tile([128, nw], i32)
            nc.vector.tensor_copy(xi32[:], ax[:])   # round-to-nearest = floor cell
            yi32 = idxp.tile([128, nw], i32)
            nc.vector.tensor_copy(yi32[:], ay[:])
            idx32 = idxp.tile([128, nw], i32)
            nc.vector.scalar_tensor_tensor(idx32[:], yi32[:], W + 1, xi32[:],
                                           OP.mult, OP.add)
            idx16 = idxp.tile([128, nw], i16)
            nc.vector.tensor_copy(idx16[:], idx32[:])

            # ---- bilinear weights (block layout [p, j] = point j*128+p) ----
            axw = wp.tile([128, NJ], f32)
            nc.scalar.activation(axw[:], pxwt[:], AT.Copy, bias=0.0, scale=inv_scale)
            ayw = wp.tile([128, NJ], f32)
            nc.scalar.activation(ayw[:], pywt[:], AT.Copy, bias=0.0, scale=inv_scale)
            xiw = wp.tile([128, NJ], i32)
            nc.vector.tensor_copy(xiw[:], axw[:])
            yiw = wp.tile([128, NJ], i32)
            nc.vector.tensor_copy(yiw[:], ayw[:])
            xif = wp.tile([128, NJ], f32)
            nc.vector.tensor_copy(xif[:], xiw[:])
            yif = wp.tile([128, NJ], f32)
            nc.vector.tensor_copy(yif[:], yiw[:])
            # fxp = (x - xi) - 0.5 in [-1, 1]; weights wx1 = fxp + 0.5, wx0 = 0.5 - fxp
            fxp = wp.tile([128, NJ], f32)
            nc.vector.scalar_tensor_tensor(fxp[:], xif[:], -1.0, axw[:],
                                           OP.mult, OP.add)
            fyp = wp.tile([128, NJ], f32)
            nc.vector.scalar_tensor_tensor(fyp[:], yif[:], -1.0, ayw[:],
                                           OP.mult, OP.add)
            wx = wp.tile([128, NJ, 2], f32)
            nc.vector.tensor_scalar(wx[:, :, 0], fxp[:], -1.0, 0.5, OP.mult, OP.add)
            nc.vector.tensor_scalar(wx[:, :, 1], fxp[:], 0.5, None, OP.add)
            wy = wp.tile([128, NJ, 2], f32)
            nc.vector.tensor_scalar(wy[:, :, 0], fyp[:], -1.0, 0.5, OP.mult, OP.add)
            nc.vector.tensor_scalar(wy[:, :, 1], fyp[:], 0.5, None, OP.add)
            # s4w[p, j, k] with k = e*2 + d (x-corner major, y fast)
            s4w = wp.tile([128, NJ, 4], f32)
            for e in range(2):
                for d in range(2):
                    nc.vector.tensor_tensor(s4w[:, :, e * 2 + d], wx[:, :, e],
                                            wy[:, :, d], OP.mult)

            # smp: bilinear result + ones column for conv bias folding.
            # float32r + padded to 32 so 4 j-columns transpose as one
            # aligned [128, 128] PE pass.
            smp = wp.tile([128, NJ, 32], f32r)
            ones_f = wp.tile([128, NJ], f32)
            nc.vector.memset(ones_f[:], 1.0)
            nc.vector.tensor_copy(smp[:, :, 16], ones_f[:])

            # ---- embed chunk emitter (interleaved with gather groups) ----
            def emit_embed(c):
                cn = min(ECH, NPTS - c)
                spc = ebuf.tile([5, ECH], f32, tag="spc")
                nc.sync.dma_start(out=spc[:, :cn], in_=sp5.ap()[:, c:c + cn])
                spcr = ebuf.tile([5, ECH], f32r, tag="spcr")
                nc.vector.tensor_copy(spcr[:, :cn], spc[:, :cn])
                pa = pse_p.tile([128, ECH], f32, tag="pe")
                nc.tensor.matmul(pa[:, :cn], wear[:], spcr[:, :cn])
                ea = ebuf.tile([128, ECH], f32, tag="ea")
                nc.vector.tensor_copy(ea[:, :cn], pa[:, :cn])
                nc.sync.dma_start(out=emb_out.ap()[0:128, c:c + cn], in_=ea[:, :cn])
                pb = pse_p.tile([128, ECH], f32, tag="pe")
                nc.tensor.matmul(pb[:, :cn], webr[:], spcr[:, :cn])
                eb = ebuf.tile([128, ECH], f32, tag="eb")
                nc.scalar.activation(eb[:, :cn], pb[:, :cn], AT.Copy,
                                     bias=0.0, scale=1.0)
                nc.sync.dma_start(out=emb_out.ap()[128:256, c:c + cn], in_=eb[:, :cn])

            emb_per_group = (NPTS // ECH + 7) // 8  # 4 chunks per group
            emb_next = 0

            # ---- main pipeline over groups of JG j-columns ----
            n_groups = (NJ + JG - 1) // JG
            for g in range(n_groups):
                j0 = g * JG
                jn = min(JG, NJ - j0)
                gt = gpool.tile([128, JG, ESZ], f32, tag="G")
                # gather this group's points (j*128+p for j in [j0, j0+jn))
                i0 = j0 * 128
                ni = jn * 128
                for c0 in range(0, ni, GCH):
                    cn = min(GCH, ni - c0)
                    nc.gpsimd.dma_gather(
                        gt[:, c0 // 128 : (c0 + cn) // 128, :],
                        feaQ.ap(),
                        idx16[:, (i0 + c0) // 16 : (i0 + c0 + cn) // 16],
                        cn, cn, ESZ)
                # weight and reduce corners
                sexp = work.tile([128, JG, 4, 16], f32, tag="sexp")
                src_b = bass.AP(
                    tensor=s4w.tensor, offset=s4w[:, j0, 0].offset,
                    ap=[s4w.ap[0], [4, jn], [1, 4], [0, 16]])
                nc.vector.tensor_copy(sexp[:, :jn], src_b)
                gw = work.tile([128, JG, 4, 16], f32, tag="gw")
                nc.vector.tensor_tensor(
                    gw[:, :jn], gt[:, :jn].rearrange("p j (k c) -> p j k c", k=4),
                    sexp[:, :jn], OP.mult)
                u2 = work.tile([128, JG, 2, 16], f32, tag="u2")
                nc.vector.tensor_tensor(u2[:, :jn], gw[:, :jn, 0:2, :],
                                        gw[:, :jn, 2:4, :], OP.add)
                nc.vector.tensor_tensor(smp[:, j0:j0 + jn, 0:16], u2[:, :jn, 0, :],
                                        u2[:, :jn, 1, :], OP.add)
                # conv: 4 j-columns per PE transpose, one f32r matmul at N=512
                for jj in range(0, jn, 4):
                    m = min(4, jn - jj)
                    pt = pst_p.tile([128, 128], f32r, tag="pt")
                    nc.tensor.transpose(
                        pt[: m * 32, :],
                        smp[:, j0 + jj : j0 + jj + m, :].rearrange(
                            "p j c -> p (j c)"),
                        identr[:])
                    stw = sbt.tile([17, 512], f32r, tag="st")
                    for q in range(m):
                        nc.vector.tensor_copy(stw[:, q * 128:(q + 1) * 128],
                                              pt[32 * q : 32 * q + 17, :])
                    pc = psc_p.tile([16, 512], f32, tag="pc")
                    nc.tensor.matmul(pc[:, : m * 128], wltr[:], stw[:, : m * 128])
                    lc = sbt.tile([16, 512], f32, tag="lc")
                    nc.scalar.activation(lc[:, : m * 128], pc[:, : m * 128],
                                         AT.Copy, bias=0.0, scale=1.0)
                    nc.sync.dma_start(
                        out=loc_out.ap()[:, (j0 + jj) * 128 : (j0 + jj + m) * 128],
                        in_=lc[:, : m * 128])
                for _ in range(emb_per_group):
                    if emb_next < NPTS:
                        emit_embed(emb_next)
                        emb_next += ECH

            while emb_next < NPTS:
                emit_embed(emb_next)
                emb_next += ECH


    nc.compile()
    return nc


def _prep_inputs(fea, sampled_3d_points, points_2d_coord, scale):
    """Host-side sharding / layout prep. Returns per-core input dicts."""
    from numpy.lib.stride_tricks import sliding_window_view

    fea = np.asarray(fea, dtype=np.float32)
    sp = np.asarray(sampled_3d_points, dtype=np.float32)
    p2 = np.asarray(points_2d_coord, dtype=np.float32)

    # quad-dup gather tables, one per batch
    feaQs = []
    for b in range(B):
        fp = np.zeros((H + 2, W + 2, C), np.float32)
        fp[1:H + 1, 1:W + 1, :] = fea[b].transpose(1, 2, 0)
        sw = sliding_window_view(fp, (2, 2), axis=(0, 1))  # [91+1.., 161.., c, d, e]
        q = sw[:H + 1, :W + 1].transpose(0, 1, 4, 3, 2)    # [yp, xp, e, d, c]
        feaQs.append(np.ascontiguousarray(q).reshape(NPOS, ESZ))

    in_maps = []
    for core in range(N_CORES):
        b = core // 2
        s0 = 4 * (core % 2)
        pts = p2[s0:s0 + 4, b].reshape(NPTS, 2)
        px = np.ascontiguousarray(pts[:, 0])
        py = np.ascontiguousarray(pts[:, 1])
        spc = sp[s0:s0 + 4, b].reshape(NPTS, 4)
        sp5 = np.empty((5, NPTS), np.float32)
        sp5[0:4] = spc.T
        sp5[4] = 1.0
        in_maps.append({
            "feaQ": feaQs[b],
            "pxw": np.ascontiguousarray(px.reshape(NJ, 128).T),
            "pyw": np.ascontiguousarray(py.reshape(NJ, 128).T),
            "pxi": np.ascontiguousarray(np.tile(px.reshape(-1, 16).T, (8, 1))),
            "pyi": np.ascontiguousarray(np.tile(py.reshape(-1, 16).T, (8, 1))),
            "sp5": sp5,
        })
    return in_maps


def kernel(fea, sampled_3d_points, points_2d_coord, W_local, b_local,
           W_emb, b_emb, scale):
    scale = float(np.asarray(scale))
    key = scale
    if key not in _cache:
        _cache[key] = _build(1.0 / scale)
    nc = _cache[key]

    W_local = np.asarray(W_local, dtype=np.float32)
    b_local = np.asarray(b_local, dtype=np.float32)
    W_emb = np.asarray(W_emb, dtype=np.float32)
    b_emb = np.asarray(b_emb, dtype=np.float32)

    wl17 = np.vstack([W_local.T, b_local[None, :]]).astype(np.float32)  # [17, 16]
    we5 = np.vstack([W_emb.T, b_emb[None, :]]).astype(np.float32)       # [5, 256]
    ident = np.eye(128, dtype=np.float32)

    in_maps = _prep_inputs(fea, sampled_3d_points, points_2d_coord, scale)
    for m in in_maps:
        m["wl17"] = wl17
        m["we5a"] = np.ascontiguousarray(we5[:, :128])
        m["we5b"] = np.ascontiguousarray(we5[:, 128:])
        m["idm"] = ident

    res = run_bass_kernel_spmd(nc, in_maps, core_ids=list(range(N_CORES)))

    embedded = np.empty((B, 256, N_FULL), np.float32)
    local = np.empty((B, 16, N_FULL), np.float32)
    for core in range(N_CORES):
        b = core // 2
        s0 = 4 * (core % 2)
        sl = slice(s0 * (N_ANCHOR * N_Y), (s0 + 4) * (N_ANCHOR * N_Y))
        embedded[b, :, sl] = res.results[core]["emb_out"]
        local[b, :, sl] = res.results[core]["loc_out"]
    return embedded, local


# revision 14
# speedup vs baseline: 1.7283x; 1.1026x over previous
"""Trainium2 Bass kernel for nn_LocalFeaLowDimDynamicFusion.

Computes, from full (unsharded) inputs:
  embedded_points = W_emb @ sampled_3d_points + b_emb   (B, 256, N)
  local_fea       = W_local @ grid_sample(fea, pts2d) + b_local  (B, 16, N)
with N = N_s * N_anchor * N_y = 32000, B = 4.

Sharding: 8 cores; core i handles batch b = i // 2 and sampling slice
s in [4*(i%2), 4*(i%2)+4) -> 16000 points per core, contiguous in the
output N axis.

Grid sample on device: the feature map is host-repacked into a
"quad-dup" table feaQ[(yp, xp), (e, d, c)] (91*161 positions x 64 f32)
so that one 256B dma_gather element fetches exactly the 4 bilinear
corner vectors of a point. Cell indices and fractional weights are
computed on DVE/ACT; the 16x16 1x1 conv runs on PE after a per-128-point
transpose, with the bias folded in via an appended ones column.
"""

import numpy as np

import concourse.bacc as bacc
import concourse.bass as bass
import concourse.tile as tile
from concourse import mybir
from concourse.bass_utils import run_bass_kernel_spmd
from concourse.library_config import mlp

B, C, H, W = 4, 16, 90, 160
N_S, N_ANCHOR, N_Y = 8, 200, 20
N_FULL = N_S * N_ANCHOR * N_Y  # 32000
N_CORES = 8
NPTS = 16000          # points per core
NJ = NPTS // 128      # 125 j-columns
NPOS = (H + 1) * (W + 1)  # 91 * 161 = 14651 gather positions
ESZ = 64              # f32 per gather element (2x2 corners x 16 ch)
GCH = 512             # gather indices per dma_gather call (ring-limit safe)
JG = 16               # j-columns per pipeline group
ECH = 500             # embed points per matmul chunk

_cache = {}


def _build(inv_scale: float):
    nc = bacc.Bacc("TRN2", target_bir_lowering=False, debug=False,
                   enable_asserts=True, num_devices=N_CORES)
    f32, i32, i16 = mybir.dt.float32, mybir.dt.int32, mybir.dt.int16
    AT = mybir.ActivationFunctionType
    OP = mybir.AluOpType

    feaQ = nc.dram_tensor("feaQ", [NPOS, ESZ], f32, kind="ExternalInput")
    pxw = nc.dram_tensor("pxw", [128, NJ], f32, kind="ExternalInput")
    pyw = nc.dram_tensor("pyw", [128, NJ], f32, kind="ExternalInput")
    pxi = nc.dram_tensor("pxi", [128, NPTS // 16], f32, kind="ExternalInput")
    pyi = nc.dram_tensor("pyi", [128, NPTS // 16], f32, kind="ExternalInput")
    sp5 = nc.dram_tensor("sp5", [5, NPTS], f32, kind="ExternalInput")
    wl17 = nc.dram_tensor("wl17", [17, 16], f32, kind="ExternalInput")
    we5a = nc.dram_tensor("we5a", [5, 128], f32, kind="ExternalInput")
    we5b = nc.dram_tensor("we5b", [5, 128], f32, kind="ExternalInput")
    idm = nc.dram_tensor("idm", [128, 128], f32, kind="ExternalInput")
    emb_out = nc.dram_tensor("emb_out", [256, NPTS], f32, kind="ExternalOutput")
    loc_out = nc.dram_tensor("loc_out", [16, NPTS], f32, kind="ExternalOutput")

    with tile.TileContext(nc) as tc:
        nc.gpsimd.load_library(mlp)
        import contextlib
        with contextlib.ExitStack() as ctx:
            const = ctx.enter_context(tc.tile_pool(name="const", bufs=1))
            idxp = ctx.enter_context(tc.tile_pool(name="idxp", bufs=1))
            wp = ctx.enter_context(tc.tile_pool(name="wp", bufs=1))
            gpool = ctx.enter_context(tc.tile_pool(name="g", bufs=2))
            work = ctx.enter_context(tc.tile_pool(name="work", bufs=2))
            sbt = ctx.enter_context(tc.tile_pool(name="sbt", bufs=3))
            ebuf = ctx.enter_context(tc.tile_pool(name="ebuf", bufs=2))
            pst_p = ctx.enter_context(tc.tile_pool(name="pst", bufs=2, space="PSUM"))
            psc_p = ctx.enter_context(tc.tile_pool(name="psc", bufs=2, space="PSUM"))
            pse_p = ctx.enter_context(tc.tile_pool(name="pse", bufs=4, space="PSUM"))

            # ---- constants / inputs to SBUF ----
            f32r = mybir.dt.float32r
            ident = const.tile([128, 128], f32)
            nc.sync.dma_start(out=ident[:], in_=idm.ap())
            identr = const.tile([128, 128], f32r)
            nc.vector.tensor_copy(identr[:], ident[:])
            wlt = const.tile([17, 16], f32)
            nc.sync.dma_start(out=wlt[:], in_=wl17.ap())
            wltr = const.tile([17, 16], f32r)
            nc.vector.tensor_copy(wltr[:], wlt[:])
            wea = const.tile([5, 128], f32)
            nc.sync.dma_start(out=wea[:], in_=we5a.ap())
            wear = const.tile([5, 128], f32r)
            nc.vector.tensor_copy(wear[:], wea[:])
            web = const.tile([5, 128], f32)
            nc.sync.dma_start(out=web[:], in_=we5b.ap())
            webr = const.tile([5, 128], f32r)
            nc.vector.tensor_copy(webr[:], web[:])
            pxwt = const.tile([128, NJ], f32)
            nc.sync.dma_start(out=pxwt[:], in_=pxw.ap())
            pywt = const.tile([128, NJ], f32)
            nc.sync.dma_start(out=pywt[:], in_=pyw.ap())
            pxit = const.tile([128, NPTS // 16], f32)
            nc.sync.dma_start(out=pxit[:], in_=pxi.ap())
            pyit = const.tile([128, NPTS // 16], f32)
            nc.sync.dma_start(out=pyit[:], in_=pyi.ap())

            # ---- gather index computation (replicated 16-wrap layout) ----
            nw = NPTS // 16  # 1000
            ax = idxp.tile([128, nw], f32)
            nc.scalar.activation(ax[:], pxit[:], AT.Copy, bias=0.0, scale=inv_scale)
            ay = idxp.tile([128, nw], f32)
            nc.scalar.activation(ay[:], pyit[:], AT.Copy, bias=0.0, scale=inv_scale)
            xi32 = idxp.tile([128, nw], i32)
            nc.vector.tensor_copy(xi32[:], ax[:])   # round-to-nearest = floor cell
            yi32 = idxp.tile([128, nw], i32)
            nc.vector.tensor_copy(yi32[:], ay[:])
            idx32 = idxp.tile([128, nw], i32)
            nc.vector.scalar_tensor_tensor(idx32[:], yi32[:], W + 1, xi32[:],
                                           OP.mult, OP.add)
            idx16 = idxp.tile([128, nw], i16)
            nc.vector.tensor_copy(idx16[:], idx32[:])

            # ---- bilinear weights (block layout [p, j] = point j*128+p) ----
            axw = wp.tile([128, NJ], f32)
            nc.scalar.activation(axw[:], pxwt[:], AT.Copy, bias=0.0, scale=inv_scale)
            ayw = wp.tile([128, NJ], f32)
            nc.scalar.activation(ayw[:], pywt[:], AT.Copy, bias=0.0, scale=inv_scale)
            xiw = wp.tile([128, NJ], i32)
            nc.vector.tensor_copy(xiw[:], axw[:])
            yiw = wp.tile([128, NJ], i32)
            nc.vector.tensor_copy(yiw[:], ayw[:])
            xif = wp.tile([128, NJ], f32)
            nc.vector.tensor_copy(xif[:], xiw[:])
            yif = wp.tile([128, NJ], f32)
            nc.vector.tensor_copy(yif[:], yiw[:])
            # fxp = (x - xi) - 0.5 in [-1, 1]; weights wx1 = fxp + 0.5, wx0 = 0.5 - fxp
            fxp = wp.tile([128, NJ], f32)
            nc.vector.scalar_tensor_tensor(fxp[:], xif[:], -1.0, axw[:],
                                           OP.mult, OP.add)
            fyp = wp.tile([128, NJ], f32)
            nc.vector.scalar_tensor_tensor(fyp[:], yif[:], -1.0, ayw[:],
                                           OP.mult, OP.add)
            wx = wp.tile([128, NJ, 2], f32)
            nc.vector.tensor_scalar(wx[:, :, 0], fxp[:], -1.0, 0.5, OP.mult, OP.add)
            nc.vector.tensor_scalar(wx[:, :, 1], fxp[:], 0.5, None, OP.add)
            wy = wp.tile([128, NJ, 2], f32)
            nc.vector.tensor_scalar(wy[:, :, 0], fyp[:], -1.0, 0.5, OP.mult, OP.add)
            nc.vector.tensor_scalar(wy[:, :, 1], fyp[:], 0.5, None, OP.add)
            # s4w[p, j, k] with k = e*2 + d (x-corner major, y fast)
            s4w = wp.tile([128, NJ, 4], f32)
            for e in range(2):
                for d in range(2):
                    nc.vector.tensor_tensor(s4w[:, :, e * 2 + d], wx[:, :, e],
                                            wy[:, :, d], OP.mult)

            # smp: bilinear result + ones column for conv bias folding.
            # float32r + padded to 32 so 4 j-columns transpose as one
            # aligned [128, 128] PE pass.
            smp = wp.tile([128, NJ, 32], f32r)
            ones_f = wp.tile([128, NJ], f32)
            nc.vector.memset(ones_f[:], 1.0)
            nc.vector.tensor_copy(smp[:, :, 16], ones_f[:])

            # ---- embed group emitter: 2000 points per call, batched DMAs ----
            EG = 2000

            def emit_embed(c0):
                gn = min(EG, NPTS - c0)
                spc = ebuf.tile([5, EG], f32, tag="spc")
                nc.sync.dma_start(out=spc[:, :gn], in_=sp5.ap()[:, c0:c0 + gn])
                spcr = ebuf.tile([5, EG], f32r, tag="spcr")
                nc.vector.tensor_copy(spcr[:, :gn], spc[:, :gn])
                ea = ebuf.tile([128, EG], f32, tag="ea")
                eb = ebuf.tile([128, EG], f32, tag="eb")
                for c in range(0, gn, ECH):
                    cn = min(ECH, gn - c)
                    pa = pse_p.tile([128, ECH], f32, tag="pe")
                    nc.tensor.matmul(pa[:, :cn], wear[:], spcr[:, c:c + cn])
                    nc.vector.tensor_copy(ea[:, c:c + cn], pa[:, :cn])
                    pb = pse_p.tile([128, ECH], f32, tag="pe")
                    nc.tensor.matmul(pb[:, :cn], webr[:], spcr[:, c:c + cn])
                    nc.scalar.activation(eb[:, c:c + cn], pb[:, :cn], AT.Copy,
                                         bias=0.0, scale=1.0)
                nc.scalar.dma_start(out=emb_out.ap()[0:128, c0:c0 + gn],
                                    in_=ea[:, :gn])
                nc.scalar.dma_start(out=emb_out.ap()[128:256, c0:c0 + gn],
                                    in_=eb[:, :gn])

            emb_next = 0

            # ---- main pipeline over groups of JG j-columns ----
            n_groups = (NJ + JG - 1) // JG
            for g in range(n_groups):
                j0 = g * JG
                jn = min(JG, NJ - j0)
                gt = gpool.tile([128, JG, ESZ], f32, tag="G")
                # gather this group's points (j*128+p for j in [j0, j0+jn))
                i0 = j0 * 128
                ni = jn * 128
                for c0 in range(0, ni, GCH):
                    cn = min(GCH, ni - c0)
                    nc.gpsimd.dma_gather(
                        gt[:, c0 // 128 : (c0 + cn) // 128, :],
                        feaQ.ap(),
                        idx16[:, (i0 + c0) // 16 : (i0 + c0 + cn) // 16],
                        cn, cn, ESZ)
                # weight and reduce corners
                sexp = work.tile([128, JG, 4, 16], f32, tag="sexp")
                src_b = bass.AP(
                    tensor=s4w.tensor, offset=s4w[:, j0, 0].offset,
                    ap=[s4w.ap[0], [4, jn], [1, 4], [0, 16]])
                nc.vector.tensor_copy(sexp[:, :jn], src_b)
                gw = work.tile([128, JG, 4, 16], f32, tag="gw")
                nc.vector.tensor_tensor(
                    gw[:, :jn], gt[:, :jn].rearrange("p j (k c) -> p j k c", k=4),
                    sexp[:, :jn], OP.mult)
                u2 = work.tile([128, JG, 2, 16], f32, tag="u2")
                nc.vector.tensor_tensor(u2[:, :jn], gw[:, :jn, 0:2, :],
                                        gw[:, :jn, 2:4, :], OP.add)
                nc.vector.tensor_tensor(smp[:, j0:j0 + jn, 0:16], u2[:, :jn, 0, :],
                                        u2[:, :jn, 1, :], OP.add)
                # conv: 4 j-columns per PE transpose, one f32r matmul at N=512,
                # whole group staged into one [16, JG*128] store
                lcw = sbt.tile([16, JG * 128], f32, tag="lcw")
                for jj in range(0, jn, 4):
                    m = min(4, jn - jj)
                    pt = pst_p.tile([128, 128], f32r, tag="pt")
                    nc.tensor.transpose(
                        pt[: m * 32, :],
                        smp[:, j0 + jj : j0 + jj + m, :].rearrange(
                            "p j c -> p (j c)"),
                        identr[:])
                    stw = sbt.tile([17, 512], f32r, tag="st")
                    for q in range(m):
                        nc.vector.tensor_copy(stw[:, q * 128:(q + 1) * 128],
                                              pt[32 * q : 32 * q + 17, :])
                    pc = psc_p.tile([16, 512], f32, tag="pc")
                    nc.tensor.matmul(pc[:, : m * 128], wltr[:], stw[:, : m * 128])
                    nc.scalar.activation(lcw[:, jj * 128 : (jj + m) * 128],
                                         pc[:, : m * 128],
                                         AT.Copy, bias=0.0, scale=1.0)
                nc.sync.dma_start(
                    out=loc_out.ap()[:, j0 * 128 : (j0 + jn) * 128],
                    in_=lcw[:, : jn * 128])
                if emb_next < NPTS:
                    emit_embed(emb_next)
                    emb_next += EG

            while emb_next < NPTS:
                emit_embed(emb_next)
                emb_next += EG


    nc.compile()
    return nc


def _prep_inputs(fea, sampled_3d_points, points_2d_coord, scale):
    """Host-side sharding / layout prep. Returns per-core input dicts."""
    from numpy.lib.stride_tricks import sliding_window_view

    fea = np.asarray(fea, dtype=np.float32)
    sp = np.asarray(sampled_3d_points, dtype=np.float32)
    p2 = np.asarray(points_2d_coord, dtype=np.float32)

    # quad-dup gather tables, one per batch
    feaQs = []
    for b in range(B):
        fp = np.zeros((H + 2, W + 2, C), np.float32)
        fp[1:H + 1, 1:W + 1, :] = fea[b].transpose(1, 2, 0)
        sw = sliding_window_view(fp, (2, 2), axis=(0, 1))  # [91+1.., 161.., c, d, e]
        q = sw[:H + 1, :W + 1].transpose(0, 1, 4, 3, 2)    # [yp, xp, e, d, c]
        feaQs.append(np.ascontiguousarray(q).reshape(NPOS, ESZ))

    in_maps = []
    for core in range(N_CORES):
        b = core // 2
        s0 = 4 * (core % 2)
        pts = p2[s0:s0 + 4, b].reshape(NPTS, 2)
        px = np.ascontiguousarray(pts[:, 0])
        py = np.ascontiguousarray(pts[:, 1])
        spc = sp[s0:s0 + 4, b].reshape(NPTS, 4)
        sp5 = np.empty((5, NPTS), np.float32)
        sp5[0:4] = spc.T
        sp5[4] = 1.0
        in_maps.append({
            "feaQ": feaQs[b],
            "pxw": np.ascontiguousarray(px.reshape(NJ, 128).T),
            "pyw": np.ascontiguousarray(py.reshape(NJ, 128).T),
            "pxi": np.ascontiguousarray(np.tile(px.reshape(-1, 16).T, (8, 1))),
            "pyi": np.ascontiguousarray(np.tile(py.reshape(-1, 16).T, (8, 1))),
            "sp5": sp5,
        })
    return in_maps


def kernel(fea, sampled_3d_points, points_2d_coord, W_local, b_local,
           W_emb, b_emb, scale):
    scale = float(np.asarray(scale))
    key = scale
    if key not in _cache:
        _cache[key] = _build(1.0 / scale)
    nc = _cache[key]

    W_local = np.asarray(W_local, dtype=np.float32)
    b_local = np.asarray(b_local, dtype=np.float32)
    W_emb = np.asarray(W_emb, dtype=np.float32)
    b_emb = np.asarray(b_emb, dtype=np.float32)

    wl17 = np.vstack([W_local.T, b_local[None, :]]).astype(np.float32)  # [17, 16]
    we5 = np.vstack([W_emb.T, b_emb[None, :]]).astype(np.float32)       # [5, 256]
    ident = np.eye(128, dtype=np.float32)

    in_maps = _prep_inputs(fea, sampled_3d_points, points_2d_coord, scale)
    for m in in_maps:
        m["wl17"] = wl17
        m["we5a"] = np.ascontiguousarray(we5[:, :128])
        m["we5b"] = np.ascontiguousarray(we5[:, 128:])
        m["idm"] = ident

    res = run_bass_kernel_spmd(nc, in_maps, core_ids=list(range(N_CORES)))

    embedded = np.empty((B, 256, N_FULL), np.float32)
    local = np.empty((B, 16, N_FULL), np.float32)
    for core in range(N_CORES):
        b = core // 2
        s0 = 4 * (core % 2)
        sl = slice(s0 * (N_ANCHOR * N_Y), (s0 + 4) * (N_ANCHOR * N_Y))
        embedded[b, :, sl] = res.results[core]["emb_out"]
        local[b, :, sl] = res.results[core]["loc_out"]
    return embedded, local


# revision 15
# speedup vs baseline: 1.7839x; 1.0322x over previous
"""Trainium2 Bass kernel for nn_LocalFeaLowDimDynamicFusion.

Computes, from full (unsharded) inputs:
  embedded_points = W_emb @ sampled_3d_points + b_emb   (B, 256, N)
  local_fea       = W_local @ grid_sample(fea, pts2d) + b_local  (B, 16, N)
with N = N_s * N_anchor * N_y = 32000, B = 4.

Sharding: 8 cores; core i handles batch b = i // 2 and sampling slice
s in [4*(i%2), 4*(i%2)+4) -> 16000 points per core, contiguous in the
output N axis.

Grid sample on device: the feature map is host-repacked into a
"quad-dup" table feaQ[(yp, xp), (e, d, c)] (91*161 positions x 64 f32)
so that one 256B dma_gather element fetches exactly the 4 bilinear
corner vectors of a point. Cell indices and fractional weights are
computed on DVE/ACT; the 16x16 1x1 conv runs on PE after a per-128-point
transpose, with the bias folded in via an appended ones column.
"""

import numpy as np

import concourse.bacc as bacc
import concourse.bass as bass
import concourse.tile as tile
from concourse import mybir
from concourse.bass_utils import run_bass_kernel_spmd
from concourse.library_config import mlp

B, C, H, W = 4, 16, 90, 160
N_S, N_ANCHOR, N_Y = 8, 200, 20
N_FULL = N_S * N_ANCHOR * N_Y  # 32000
N_CORES = 8
NPTS = 16000          # points per core
NJ = NPTS // 128      # 125 j-columns
NPOS = (H + 1) * (W + 1)  # 91 * 161 = 14651 gather positions
ESZ = 64              # f32 per gather element (2x2 corners x 16 ch)
GCH = 512             # gather indices per dma_gather call (ring-limit safe)
JG = 16               # j-columns per pipeline group
ECH = 500             # embed points per matmul chunk

_cache = {}


def _build(inv_scale: float):
    nc = bacc.Bacc("TRN2", target_bir_lowering=False, debug=False,
                   enable_asserts=True, num_devices=N_CORES)
    f32, i32, i16 = mybir.dt.float32, mybir.dt.int32, mybir.dt.int16
    AT = mybir.ActivationFunctionType
    OP = mybir.AluOpType

    feaQ = nc.dram_tensor("feaQ", [NPOS, ESZ], f32, kind="ExternalInput")
    pxw = nc.dram_tensor("pxw", [128, NJ], f32, kind="ExternalInput")
    pyw = nc.dram_tensor("pyw", [128, NJ], f32, kind="ExternalInput")
    pxi = nc.dram_tensor("pxi", [128, NPTS // 16], f32, kind="ExternalInput")
    pyi = nc.dram_tensor("pyi", [128, NPTS // 16], f32, kind="ExternalInput")
    sp5 = nc.dram_tensor("sp5", [5, NPTS], f32, kind="ExternalInput")
    wl17 = nc.dram_tensor("wl17", [17, 16], f32, kind="ExternalInput")
    we5a = nc.dram_tensor("we5a", [5, 128], f32, kind="ExternalInput")
    we5b = nc.dram_tensor("we5b", [5, 128], f32, kind="ExternalInput")
    idm = nc.dram_tensor("idm", [128, 128], f32, kind="ExternalInput")
    emb_out = nc.dram_tensor("emb_out", [256, NPTS], f32, kind="ExternalOutput")
    loc_out = nc.dram_tensor("loc_out", [16, NPTS], f32, kind="ExternalOutput")

    with tile.TileContext(nc) as tc:
        nc.gpsimd.load_library(mlp)
        import contextlib
        with contextlib.ExitStack() as ctx:
            const = ctx.enter_context(tc.tile_pool(name="const", bufs=1))
            idxp = ctx.enter_context(tc.tile_pool(name="idxp", bufs=1))
            wp = ctx.enter_context(tc.tile_pool(name="wp", bufs=1))
            gpool = ctx.enter_context(tc.tile_pool(name="g", bufs=2))
            work = ctx.enter_context(tc.tile_pool(name="work", bufs=2))
            sbt = ctx.enter_context(tc.tile_pool(name="sbt", bufs=3))
            ebuf = ctx.enter_context(tc.tile_pool(name="ebuf", bufs=2))
            pst_p = ctx.enter_context(tc.tile_pool(name="pst", bufs=2, space="PSUM"))
            psc_p = ctx.enter_context(tc.tile_pool(name="psc", bufs=2, space="PSUM"))
            pse_p = ctx.enter_context(tc.tile_pool(name="pse", bufs=4, space="PSUM"))

            # ---- constants / inputs to SBUF ----
            f32r = mybir.dt.float32r
            ident = const.tile([128, 128], f32)
            nc.sync.dma_start(out=ident[:], in_=idm.ap())
            identr = const.tile([128, 128], f32r)
            nc.vector.tensor_copy(identr[:], ident[:])
            wlt = const.tile([17, 16], f32)
            nc.sync.dma_start(out=wlt[:], in_=wl17.ap())
            wltr = const.tile([17, 16], f32r)
            nc.vector.tensor_copy(wltr[:], wlt[:])
            wea = const.tile([5, 128], f32)
            nc.sync.dma_start(out=wea[:], in_=we5a.ap())
            wear = const.tile([5, 128], f32r)
            nc.vector.tensor_copy(wear[:], wea[:])
            web = const.tile([5, 128], f32)
            nc.sync.dma_start(out=web[:], in_=we5b.ap())
            webr = const.tile([5, 128], f32r)
            nc.vector.tensor_copy(webr[:], web[:])
            pxwt = const.tile([128, NJ], f32)
            nc.sync.dma_start(out=pxwt[:], in_=pxw.ap())
            pywt = const.tile([128, NJ], f32)
            nc.sync.dma_start(out=pywt[:], in_=pyw.ap())
            pxit = const.tile([128, NPTS // 16], f32)
            nc.sync.dma_start(out=pxit[:], in_=pxi.ap())
            pyit = const.tile([128, NPTS // 16], f32)
            nc.sync.dma_start(out=pyit[:], in_=pyi.ap())

            # ---- gather index computation (replicated 16-wrap layout) ----
            # split so group 0's indices (first 128 wrap-cols = 2048 points)
            # are ready before the rest of the chain runs
            nw = NPTS // 16  # 1000
            NA = JG * 8      # wrap-cols covering group 0
            idx_tiles = []
            for (lo, hi, tg) in ((0, NA, "a"), (NA, nw, "b")):
                n = hi - lo
                ax = idxp.tile([128, n], f32, tag="ax" + tg)
                nc.scalar.activation(ax[:], pxit[:, lo:hi], AT.Copy,
                                     bias=0.0, scale=inv_scale)
                ay = idxp.tile([128, n], f32, tag="ay" + tg)
                nc.scalar.activation(ay[:], pyit[:, lo:hi], AT.Copy,
                                     bias=0.0, scale=inv_scale)
                xi32 = idxp.tile([128, n], i32, tag="xi" + tg)
                nc.vector.tensor_copy(xi32[:], ax[:])  # round-to-nearest cell
                yi32 = idxp.tile([128, n], i32, tag="yi" + tg)
                nc.vector.tensor_copy(yi32[:], ay[:])
                idx32 = idxp.tile([128, n], i32, tag="ix" + tg)
                nc.vector.scalar_tensor_tensor(idx32[:], yi32[:], W + 1, xi32[:],
                                               OP.mult, OP.add)
                idx16_t = idxp.tile([128, n], i16, tag="i16" + tg)
                nc.vector.tensor_copy(idx16_t[:], idx32[:])
                idx_tiles.append((lo, hi, idx16_t))

            def idx_slice(c0w, c1w):
                for lo, hi, t in idx_tiles:
                    if c0w >= lo and c1w <= hi:
                        return t[:, c0w - lo : c1w - lo]
                raise AssertionError((c0w, c1w))

            # ---- bilinear weights (block layout [p, j] = point j*128+p) ----
            axw = wp.tile([128, NJ], f32)
            nc.scalar.activation(axw[:], pxwt[:], AT.Copy, bias=0.0, scale=inv_scale)
            ayw = wp.tile([128, NJ], f32)
            nc.scalar.activation(ayw[:], pywt[:], AT.Copy, bias=0.0, scale=inv_scale)
            xiw = wp.tile([128, NJ], i32)
            nc.vector.tensor_copy(xiw[:], axw[:])
            yiw = wp.tile([128, NJ], i32)
            nc.vector.tensor_copy(yiw[:], ayw[:])
            xif = wp.tile([128, NJ], f32)
            nc.vector.tensor_copy(xif[:], xiw[:])
            yif = wp.tile([128, NJ], f32)
            nc.vector.tensor_copy(yif[:], yiw[:])
            # fxp = (x - xi) - 0.5 in [-1, 1]; weights wx1 = fxp + 0.5, wx0 = 0.5 - fxp
            fxp = wp.tile([128, NJ], f32)
            nc.vector.scalar_tensor_tensor(fxp[:], xif[:], -1.0, axw[:],
                                           OP.mult, OP.add)
            fyp = wp.tile([128, NJ], f32)
            nc.vector.scalar_tensor_tensor(fyp[:], yif[:], -1.0, ayw[:],
                                           OP.mult, OP.add)
            wx = wp.tile([128, NJ, 2], f32)
            nc.vector.tensor_scalar(wx[:, :, 0], fxp[:], -1.0, 0.5, OP.mult, OP.add)
            nc.vector.tensor_scalar(wx[:, :, 1], fxp[:], 0.5, None, OP.add)
            wy = wp.tile([128, NJ, 2], f32)
            nc.vector.tensor_scalar(wy[:, :, 0], fyp[:], -1.0, 0.5, OP.mult, OP.add)
            nc.vector.tensor_scalar(wy[:, :, 1], fyp[:], 0.5, None, OP.add)
            # s4w[p, j, k] with k = e*2 + d (x-corner major, y fast)
            s4w = wp.tile([128, NJ, 4], f32)
            for e in range(2):
                for d in range(2):
                    nc.vector.tensor_tensor(s4w[:, :, e * 2 + d], wx[:, :, e],
                                            wy[:, :, d], OP.mult)

            # smp: bilinear result + ones column for conv bias folding.
            # float32r + padded to 32 so 4 j-columns transpose as one
            # aligned [128, 128] PE pass.
            smp = wp.tile([128, NJ, 32], f32r)
            ones_f = wp.tile([128, NJ], f32)
            nc.vector.memset(ones_f[:], 1.0)
            nc.vector.tensor_copy(smp[:, :, 16], ones_f[:])

            # ---- embed group emitter: 2000 points per call, batched DMAs ----
            EG = 2000

            def emit_embed(c0):
                gn = min(EG, NPTS - c0)
                spc = ebuf.tile([5, EG], f32, tag="spc")
                nc.sync.dma_start(out=spc[:, :gn], in_=sp5.ap()[:, c0:c0 + gn])
                spcr = ebuf.tile([5, EG], f32r, tag="spcr")
                nc.vector.tensor_copy(spcr[:, :gn], spc[:, :gn])
                ea = ebuf.tile([128, EG], f32, tag="ea")
                eb = ebuf.tile([128, EG], f32, tag="eb")
                for c in range(0, gn, ECH):
                    cn = min(ECH, gn - c)
                    pa = pse_p.tile([128, ECH], f32, tag="pe")
                    nc.tensor.matmul(pa[:, :cn], wear[:], spcr[:, c:c + cn])
                    nc.vector.tensor_copy(ea[:, c:c + cn], pa[:, :cn])
                    pb = pse_p.tile([128, ECH], f32, tag="pe")
                    nc.tensor.matmul(pb[:, :cn], webr[:], spcr[:, c:c + cn])
                    nc.scalar.activation(eb[:, c:c + cn], pb[:, :cn], AT.Copy,
                                         bias=0.0, scale=1.0)
                nc.scalar.dma_start(out=emb_out.ap()[0:128, c0:c0 + gn],
                                    in_=ea[:, :gn])
                nc.scalar.dma_start(out=emb_out.ap()[128:256, c0:c0 + gn],
                                    in_=eb[:, :gn])

            emb_next = 0

            # ---- main pipeline over groups of JG j-columns ----
            n_groups = (NJ + JG - 1) // JG
            for g in range(n_groups):
                j0 = g * JG
                jn = min(JG, NJ - j0)
                gt = gpool.tile([128, JG, ESZ], f32, tag="G")
                # gather this group's points (j*128+p for j in [j0, j0+jn))
                i0 = j0 * 128
                ni = jn * 128
                for c0 in range(0, ni, GCH):
                    cn = min(GCH, ni - c0)
                    nc.gpsimd.dma_gather(
                        gt[:, c0 // 128 : (c0 + cn) // 128, :],
                        feaQ.ap(),
                        idx_slice((i0 + c0) // 16, (i0 + c0 + cn) // 16),
                        cn, cn, ESZ)
                # weight and reduce corners
                sexp = work.tile([128, JG, 4, 16], f32, tag="sexp")
                src_b = bass.AP(
                    tensor=s4w.tensor, offset=s4w[:, j0, 0].offset,
                    ap=[s4w.ap[0], [4, jn], [1, 4], [0, 16]])
                nc.vector.tensor_copy(sexp[:, :jn], src_b)
                gw = work.tile([128, JG, 4, 16], f32, tag="gw")
                nc.vector.tensor_tensor(
                    gw[:, :jn], gt[:, :jn].rearrange("p j (k c) -> p j k c", k=4),
                    sexp[:, :jn], OP.mult)
                u2 = work.tile([128, JG, 2, 16], f32, tag="u2")
                nc.vector.tensor_tensor(u2[:, :jn], gw[:, :jn, 0:2, :],
                                        gw[:, :jn, 2:4, :], OP.add)
                nc.vector.tensor_tensor(smp[:, j0:j0 + jn, 0:16], u2[:, :jn, 0, :],
                                        u2[:, :jn, 1, :], OP.add)
                # conv: 4 j-columns per PE transpose, one f32r matmul at N=512,
                # whole group staged into one [16, JG*128] store
                lcw = sbt.tile([16, JG * 128], f32, tag="lcw")
                for jj in range(0, jn, 4):
                    m = min(4, jn - jj)
                    pt = pst_p.tile([128, 128], f32r, tag="pt")
                    nc.tensor.transpose(
                        pt[: m * 32, :],
                        smp[:, j0 + jj : j0 + jj + m, :].rearrange(
                            "p j c -> p (j c)"),
                        identr[:])
                    stw = sbt.tile([17, 512], f32r, tag="st")
                    for q in range(m):
                        nc.vector.tensor_copy(stw[:, q * 128:(q + 1) * 128],
                                              pt[32 * q : 32 * q + 17, :])
                    pc = psc_p.tile([16, 512], f32, tag="pc")
                    nc.tensor.matmul(pc[:, : m * 128], wltr[:], stw[:, : m * 128])
                    nc.scalar.activation(lcw[:, jj * 128 : (jj + m) * 128],
                                         pc[:, : m * 128],
                                         AT.Copy, bias=0.0, scale=1.0)
                nc.sync.dma_start(
                    out=loc_out.ap()[:, j0 * 128 : (j0 + jn) * 128],
                    in_=lcw[:, : jn * 128])
                if emb_next < NPTS:
                    emit_embed(emb_next)
                    emb_next += EG

            while emb_next < NPTS:
                emit_embed(emb_next)
                emb_next += EG


    nc.compile()
    return nc


def _prep_inputs(fea, sampled_3d_points, points_2d_coord, scale):
    """Host-side sharding / layout prep. Returns per-core input dicts."""
    from numpy.lib.stride_tricks import sliding_window_view

    fea = np.asarray(fea, dtype=np.float32)
    sp = np.asarray(sampled_3d_points, dtype=np.float32)
    p2 = np.asarray(points_2d_coord, dtype=np.float32)

    # quad-dup gather tables, one per batch
    feaQs = []
    for b in range(B):
        fp = np.zeros((H + 2, W + 2, C), np.float32)
        fp[1:H + 1, 1:W + 1, :] = fea[b].transpose(1, 2, 0)
        sw = sliding_window_view(fp, (2, 2), axis=(0, 1))  # [91+1.., 161.., c, d, e]
        q = sw[:H + 1, :W + 1].transpose(0, 1, 4, 3, 2)    # [yp, xp, e, d, c]
        feaQs.append(np.ascontiguousarray(q).reshape(NPOS, ESZ))

    in_maps = []
    for core in range(N_CORES):
        b = core // 2
        s0 = 4 * (core % 2)
        pts = p2[s0:s0 + 4, b].reshape(NPTS, 2)
        px = np.ascontiguousarray(pts[:, 0])
        py = np.ascontiguousarray(pts[:, 1])
        spc = sp[s0:s0 + 4, b].reshape(NPTS, 4)
        sp5 = np.empty((5, NPTS), np.float32)
        sp5[0:4] = spc.T
        sp5[4] = 1.0
        in_maps.append({
            "feaQ": feaQs[b],
            "pxw": np.ascontiguousarray(px.reshape(NJ, 128).T),
            "pyw": np.ascontiguousarray(py.reshape(NJ, 128).T),
            "pxi": np.ascontiguousarray(np.tile(px.reshape(-1, 16).T, (8, 1))),
            "pyi": np.ascontiguousarray(np.tile(py.reshape(-1, 16).T, (8, 1))),
            "sp5": sp5,
        })
    return in_maps


def kernel(fea, sampled_3d_points, points_2d_coord, W_local, b_local,
           W_emb, b_emb, scale):
    scale = float(np.asarray(scale))
    key = scale
    if key not in _cache:
        _cache[key] = _build(1.0 / scale)
    nc = _cache[key]

    W_local = np.asarray(W_local, dtype=np.float32)
    b_local = np.asarray(b_local, dtype=np.float32)
    W_emb = np.asarray(W_emb, dtype=np.float32)
    b_emb = np.asarray(b_emb, dtype=np.float32)

    wl17 = np.vstack([W_local.T, b_local[None, :]]).astype(np.float32)  # [17, 16]
    we5 = np.vstack([W_emb.T, b_emb[None, :]]).astype(np.float32)       # [5, 256]
    ident = np.eye(128, dtype=np.float32)

    in_maps = _prep_inputs(fea, sampled_3d_points, points_2d_coord, scale)
    for m in in_maps:
        m["wl17"] = wl17
        m["we5a"] = np.ascontiguousarray(we5[:, :128])
        m["we5b"] = np.ascontiguousarray(we5[:, 128:])
        m["idm"] = ident

    res = run_bass_kernel_spmd(nc, in_maps, core_ids=list(range(N_CORES)))

    embedded = np.empty((B, 256, N_FULL), np.float32)
    local = np.empty((B, 16, N_FULL), np.float32)
    for core in range(N_CORES):
        b = core // 2
        s0 = 4 * (core % 2)
        sl = slice(s0 * (N_ANCHOR * N_Y), (s0 + 4) * (N_ANCHOR * N_Y))
        embedded[b, :, sl] = res.results[core]["emb_out"]
        local[b, :, sl] = res.results[core]["loc_out"]
    return embedded, local
